# revision 2
# baseline (speedup 1.0000x reference)
"""FlowNet correlation kernel v2 for Trainium2 (8 NeuronCores, SPMD over batch).

out[b, d, y, x] = (1/C) * sum_c in1[b,c,y,x] * in2pad[b,c,y+dy,x+dx],
d = dy*9+dx, displacements in [-4,4]^2, B=8, C=128, H=W=192.

v2 strategy (per core = one batch element):
  - Host pre-casts inputs to bf16 and pre-pads x2 to [C, 200, 200].
  - PE column tiling: 4 concurrent 32-wide column tiles (tile_position),
    each one x-column of 32 y-pixels vs its own [40, 9] window (N=360).
  - Band layout per psum row m=32j+py: n = 9*py + d -> 81 displacements
    of a pixel are contiguous.
  - 4 x-groups share a 4-bank PSUM mega tile; one strided DVE/ACT copy
    drains 4 banks -> bf16 band arena (manual double-buffered SBUF).
  - Diagonal extraction: per column-tile j one 3-dim DMA
    [[BROW+9, 32], [360, 24], [1, 81]] from an aliased 32-partition view
    of the band arena at base_partition=32j (walrus requires intra-row
    offsets for non-clean partition steps) -> DRAM out[pixel, d] bf16.
  - Aliasing is invisible to Tile's dep tracker -> explicit
    add_dep_helper sync edges: gathers after copies, next copies into an
    arena after the previous gathers from it.
  - Host converts bf16 -> fp32 and applies the 1/C scale.
"""
import sys

import numpy as np

B, C, H, W = 8, 128, 192, 192
MD = 4
D = 81
PW = W + 2 * MD          # 200
PH = H + 2 * MD          # 200
YC = 32                  # rows per y-chunk / column tile
NYC = H // YC            # 6
NW = 40                  # window rows (YC + 8)
NB = NW * 9              # 360
XG = 4                   # columns per x-group (4 concurrent col tiles)
GH = 24                  # x-groups per half-row
MG = 4                   # x-groups per PSUM mega tile
BROW = GH * NB           # 8640 band elems per partition per half
INV_C = 1.0 / C

_CACHE: dict = {}


def _build():
    sys.path.insert(0, "/opt/trn_rl_repo")
    from contextlib import ExitStack

    import concourse.bacc as bacc
    import concourse.bass as bass
    import concourse.tile as tile
    from concourse.bass_types import SBTensorHandle
    from concourse.tile import add_dep_helper
    from concourse import mybir

    BF16 = mybir.dt.bfloat16
    F32 = mybir.dt.float32

    nc = bacc.Bacc()
    x1_d = nc.declare_dram_parameter("x1", [C, H, W], BF16, isOutput=False)
    x2_d = nc.declare_dram_parameter("x2p", [C, PH, PW], BF16, isOutput=False)
    out_d = nc.declare_dram_parameter("out", [H * W, D], BF16, isOutput=True)

    # manual double-buffered band arenas + aliased per-column-tile views
    arenas = []
    for a in range(2):
        band = nc.alloc_sbuf_tensor(f"band{a}", [128, BROW], BF16)
        mloc = nc.lookup_mloc(band)
        views = [band]
        for j in range(1, XG):
            vname = f"band{a}v{j}"
            nc._tensor(vname, [32, BROW], BF16, type="SB")
            v = SBTensorHandle(vname, [32, BROW], BF16, base_partition=32 * j)
            vm = nc.lookup_mloc(v)
            vm.addr = mloc.addr
            vm.base = 32 * j
            vm.allocated = True
            views.append(v)
        arenas.append((band, views))

    with tile.TileContext(nc) as tc, ExitStack() as ctx:
        p_x2 = ctx.enter_context(tc.tile_pool(name="p_x2", bufs=1))
        p_x1 = ctx.enter_context(tc.tile_pool(name="p_x1", bufs=2))
        p_ps = ctx.enter_context(tc.tile_pool(name="p_ps", bufs=2, space="PSUM"))

        x2s = p_x2.tile([C, PH * PW], BF16, tag="x2s")
        row_chunks = [(0, NW)] + [(NW + 32 * i, NW + 32 * (i + 1)) for i in range(5)]
        for r0, r1 in row_chunks:
            nc.gpsimd.dma_start(
                out=bass.AP(x2s[:].tensor, x2s[:].offset + r0 * PW,
                            [x2s[:].ap[0], [1, (r1 - r0) * PW]]),
                in_=x2_d[:, r0:r1, :],
            )

        x1c = {}
        for yc in range(NYC):
            x1c[yc] = p_x1.tile([C, YC * W], BF16, tag="x1c", name=f"x1c{yc}")
            nc.gpsimd.dma_start(out=x1c[yc][:], in_=x1_d[:, yc * YC : (yc + 1) * YC, :])

        copy_tick = 0
        prev_gathers = {0: [], 1: []}   # arena -> gather insts (for reuse fencing)
        for yc in range(NYC):
            for half in range(2):
                ai = (yc * 2 + half) % 2
                band, views = arenas[ai]
                copies = []
                for mg in range(GH // MG):
                    ps = p_ps.tile([128, 2048], F32, tag="ps", name=f"ps{yc}_{half}_{mg}")
                    for gl in range(MG):
                        xg = half * GH + mg * MG + gl
                        for j in range(XG):
                            x = XG * xg + j
                            lhsT = bass.AP(x1c[yc][:].tensor, x1c[yc][:].offset + x,
                                           [x1c[yc][:].ap[0], [W, YC]])
                            rhs = bass.AP(x2s[:].tensor,
                                          x2s[:].offset + (yc * YC) * PW + x,
                                          [x2s[:].ap[0], [PW, NW], [1, 9]])
                            nc.tensor.matmul(
                                ps[32 * j : 32 * (j + 1),
                                   512 * gl : 512 * gl + NB],
                                lhsT, rhs, start=True, stop=True,
                                tile_position=(0, 32 * j),
                            )
                    src = bass.AP(ps[:].tensor, ps[:].offset,
                                  [ps[:].ap[0], [512, MG], [1, NB]])
                    dst = bass.AP(band, mg * MG * NB,
                                  [[BROW, 128], [NB, MG], [1, NB]])
                    if copy_tick % 2 == 0:
                        cp = nc.vector.tensor_copy(dst, src)
                    else:
                        cp = nc.scalar.copy(dst, src)
                    copy_tick += 1
                    copies.append(cp)
                    # fence arena reuse: this copy must follow the gathers
                    # that last read this arena (2 iterations ago)
                    for g in prev_gathers[ai]:
                        add_dep_helper(cp.ins, g.ins, sync=True,
                                       reason="band arena reuse WAR")

                gathers = []
                for j in range(XG):
                    gsrc = bass.AP(views[j], 0,
                                   [[BROW + 9, YC], [NB, GH], [1, D]])
                    gdst = bass.AP(out_d[:].tensor,
                                   ((yc * YC) * W + half * 96 + j) * D,
                                   [[W * D, YC], [XG * D, GH], [1, D]])
                    g = nc.sync.dma_start(out=gdst, in_=gsrc)
                    # aliased views are invisible to dep tracking: order
                    # every gather after every copy of this half explicitly
                    for cp in copies:
                        add_dep_helper(g.ins, cp.ins, sync=True,
                                       reason="gather RAW on band copies")
                    gathers.append(g)
                prev_gathers[ai] = gathers

    nc.compile()
    return nc


def _get_nc():
    if "nc" not in _CACHE:
        _CACHE["nc"] = _build()
    return _CACHE["nc"]


def _prep_core(x1_f32: np.ndarray, x2_f32: np.ndarray) -> dict:
    import ml_dtypes

    bf16 = ml_dtypes.bfloat16
    x1 = np.ascontiguousarray(x1_f32.astype(bf16))
    x2p = np.zeros((C, PH, PW), dtype=bf16)
    x2p[:, MD : MD + H, MD : MD + W] = x2_f32.astype(bf16)
    return {"x1": x1, "x2p": x2p}


def kernel(input1: np.ndarray, input2: np.ndarray) -> np.ndarray:
    sys.path.insert(0, "/opt/trn_rl_repo")
    from concourse.bass_utils import run_bass_kernel_spmd

    nc = _get_nc()
    input1 = np.asarray(input1, dtype=np.float32)
    input2 = np.asarray(input2, dtype=np.float32)
    in_maps = [_prep_core(input1[i], input2[i]) for i in range(B)]
    res = run_bass_kernel_spmd(nc, in_maps, core_ids=list(range(B)))
    outs = np.stack(
        [np.asarray(res.results[i]["out"], dtype=np.float32) for i in range(B)]
    )
    out = outs.reshape(B, H, W, D).transpose(0, 3, 1, 2)
    return np.ascontiguousarray(out * np.float32(INV_C))
